# revision 31
# baseline (speedup 1.0000x reference)
"""CompGCN (3-layer) Trainium2 Bass kernel, 8-core SPMD.

Strategy:
  - Nodes are dst-sharded: core c owns nodes [c*12500, (c+1)*12500).
  - Per layer, each core gathers dinv-prescaled source rows (x~ = dinv_dir * x,
    bf16 padded to 512B rows) for the edges landing in its shard via indirect
    DMA, reduces them into per-dst-tile aggregates with one-hot matmuls whose
    dst masks carry the dinv_dst/3 norm factor (PSUM accumulation), then runs
    the (tiny) dense W matmuls feature-major in a single PSUM accumulation
    chain, tanh(+relu), and AllGathers the new x~ buffers.
  - The relation correction uses a type histogram M'[type, dst] =
    sum_e dinv_src*dinv_dst/3 built ON DEVICE once (layer-invariant) from
    per-slot type/weight metadata via the same one-hot matmul machinery,
    staged in a DRAM scratch tile, and applied per layer as a dense matmul
    against rel@W.
  - Final graph mean-pool + linear head also run on device; partial pooled
    sums are AllReduced.

Host-side work is limited to index/layout derivations (edge sorting, degree/
norm factors, per-slot metadata) - all FLOPs on data tensors happen on
device. The executor (jitted shard_map over 8 cores) is built and compiled
once per process and cached; per-call cost is host prep + input upload +
device execution.
"""

import sys
import math
from dataclasses import dataclass

import numpy as np

sys.path.insert(0, "/opt/trn_rl_repo")

import ml_dtypes  # noqa: E402

P = 128
H = 128
PAD_ID = 0  # pad slots gather row 0; their mask column is 0 so they add nothing


@dataclass
class Cfg:
    n_nodes: int = 100000
    n_edges: int = 1000000  # total (half in, half out)
    n_cores: int = 8
    n_graphs: int = 256
    n_rel: int = 200      # rel_labels vocabulary (embedding table rows)
    n_relg: int = 100     # edge_type in [0, 2*n_relg)
    row_pad: int = 256    # x~ row width in elems (bf16 -> 512B rows)
    tiles_per_gather: int = 2

    @property
    def nloc(self):
        return self.n_nodes // self.n_cores

    @property
    def nt(self):  # node tiles per core
        return (self.nloc + P - 1) // P

    @property
    def nlp(self):  # padded local nodes
        return self.nt * P

    @property
    def n_types(self):
        return 2 * self.n_relg


def _f32(x):
    return np.ascontiguousarray(x, dtype=np.float32)


def _bf16(x):
    return np.ascontiguousarray(np.asarray(x, dtype=np.float32).astype(ml_dtypes.bfloat16))


def _fast_bf16(a):
    """f32 -> bf16 via integer round-to-nearest (ties-away); ~10x faster than
    ml_dtypes astype for large arrays."""
    a = np.ascontiguousarray(a, dtype=np.float32)
    u = a.view(np.uint32) + 0x8000
    np.right_shift(u, 16, out=u)
    r = u.astype(np.uint16)
    return r.view(ml_dtypes.bfloat16).reshape(a.shape)


def host_prepare_cat(inputs, cfg: Cfg):
    """Vectorized preprocessing. Returns a dict of CONCATENATED (axis 0 over
    cores) input arrays keyed by kernel input name, plus spt."""
    C = cfg.n_cores
    N = cfg.n_nodes
    E = cfg.n_edges
    nloc, nlp, nt = cfg.nloc, cfg.nlp, cfg.nt
    half = E // 2

    edge_index = np.asarray(inputs["edge_index"])
    edge_type = np.asarray(inputs["edge_type"])
    batch = np.asarray(inputs["batch"])
    rel_labels = np.asarray(inputs["rel_labels"])
    x = np.asarray(inputs["x"], dtype=np.float32)

    # --- per-direction edge data (the two directions run in parallel;
    # numpy's sort/astype kernels release the GIL) ---------------------------
    def _dir_prep(d):
        sl = slice(0, half) if d == 0 else slice(half, E)
        src = edge_index[0, sl].astype(np.int32)
        dst = edge_index[1, sl].astype(np.int32)
        et = edge_type[sl].astype(np.int32)
        deg = np.bincount(src, minlength=N)
        dinv = np.zeros(N, np.float32)
        nz = deg > 0
        dinv[nz] = deg[nz].astype(np.float64) ** -0.5
        core, loc = np.divmod(dst, np.int32(nloc))
        tile, rel = np.divmod(loc, np.int32(P))
        key = core * np.int32(nt) + tile
        order = np.argsort(key, kind="stable")
        ks = key[order]
        starts = np.searchsorted(ks, np.arange(C * nt), side="left")
        rank = (np.arange(half, dtype=np.int32) - starts[ks]).astype(np.int32)
        cmax = int(np.diff(np.append(starts, half)).max())
        return (src[order], et[order], dst[order], rel[order],
                tile[order], core[order], rank, dinv), cmax

    from concurrent.futures import ThreadPoolExecutor
    with ThreadPoolExecutor(2) as ex:
        res = list(ex.map(_dir_prep, range(2)))
    dir_data = [r[0] for r in res]
    counts_max = max(r[1] for r in res)

    spt = int(math.ceil(counts_max / P))

    # --- per-slot edge metadata --------------------------------------------
    gidx = np.zeros((C, P, 2 * nt * spt), dtype=np.int32)  # PAD_ID = 0
    dstrel_f = np.full((C, P, 2 * nt * spt), 255.0, dtype=np.float32)
    styp_f = np.full((C, P, 2 * nt * spt), 300.0, dtype=np.float32)
    dvalsrc_f = np.zeros((C, P, 2 * nt * spt), dtype=np.float32)
    dval3_f = np.zeros((C, P, 2 * nt * spt), dtype=np.float32)
    ncols = 2 * nt * spt
    for d in range(2):
        src_s, et_s, dst_s, rel_s, tile_s, core_s, rank, dinv = dir_data[d]
        part = rank % np.int32(P)
        col = (np.int32(d * nt * spt) + tile_s * np.int32(spt)
               + rank // np.int32(P))
        sc, lo = np.divmod(src_s, np.int32(nloc))
        g = sc * np.int32(2 * nlp) + np.int32(d * nlp) + lo
        fi = (core_s * np.int32(P) + part) * np.int32(ncols) + col
        gidx.ravel()[fi] = g
        dstrel_f.ravel()[fi] = rel_s
        styp_f.ravel()[fi] = et_s
        dvalsrc_f.ravel()[fi] = dinv[src_s]
        dval3_f.ravel()[fi] = dinv[dst_s] * np.float32(1.0 / 3.0)
    dstrel = _fast_bf16(dstrel_f)
    styp = _fast_bf16(styp_f)
    dvalsrc = _fast_bf16(dvalsrc_f)
    dval3 = _fast_bf16(dval3_f)

    # --- node-sharded tensors ----------------------------------------------
    x_shard = np.zeros((C, nlp, H), dtype=ml_dtypes.bfloat16)
    x_shard[:, :nloc] = _fast_bf16(x).reshape(C, nloc, H)

    dinv_own = np.zeros((C, 2, P, nt), dtype=np.float32)
    for d in range(2):
        dinv = dir_data[d][7]
        dvp = np.zeros((C, nlp), np.float64)
        dvp[:, :nloc] = dinv.reshape(C, nloc)
        dinv_own[:, d] = np.transpose(dvp.reshape(C, nt, P), (0, 2, 1))

    batch_pad = np.full((C, nlp), 300.0, dtype=np.float32)
    batch_pad[:, :nloc] = batch.reshape(C, nloc)
    batchrel = _fast_bf16(np.transpose(batch_pad.reshape(C, nt, P), (0, 2, 1)))

    cnt = np.bincount(batch, minlength=cfg.n_graphs).astype(np.float64)
    invcnt = (1.0 / np.maximum(cnt, 1.0)).astype(np.float32)
    g_pad = 2 * P
    invcnt_a = np.zeros((g_pad,), np.float32)
    invcnt_a[: cfg.n_graphs] = invcnt
    invcnt_pp = np.ascontiguousarray(invcnt_a.reshape(2, P).transpose(1, 0))

    onehotRT = np.zeros((P, 512), dtype=np.float32)
    t = np.asarray(rel_labels).astype(np.int64)
    onehotRT[t % P, (t // P) * 256 + np.arange(cfg.n_graphs)] = 1.0

    def rep(a):
        a = np.ascontiguousarray(a)
        return np.ascontiguousarray(
            np.broadcast_to(a, (C,) + a.shape)
        ).reshape((C * a.shape[0],) + a.shape[1:])

    cat = {
        "x_shard": x_shard.reshape(C * nlp, H),
        "gidx": gidx.reshape(C * P, -1),
        "dstrel": dstrel.reshape(C * P, -1),
        "styp": styp.reshape(C * P, -1),
        "dvalsrc": dvalsrc.reshape(C * P, -1),
        "dval3": dval3.reshape(C * P, -1),
        "dinv_own": dinv_own.reshape(C * 2, P, nt),
        "batchrel": batchrel.reshape(C * P, nt),
        "invcnt": rep(invcnt_pp),
        "onehotRT": rep(onehotRT),
        "rgT": rep(_f32(np.asarray(inputs["rel_graph_emb"]).T)),
        "tableT": rep(_f32(np.asarray(inputs["rel_emb_table"]).T)),
        "lin1": rep(_f32(np.asarray(inputs["lin_w"])[:H])),
        "lin2": rep(_f32(np.asarray(inputs["lin_w"])[H:])),
        "lin_b": rep(_f32(np.asarray(inputs["lin_b"]).reshape(1, 2))),
    }
    for l in (1, 2, 3):
        for nm in ("w_in", "w_out", "w_loop", "w_rel"):
            cat[f"{nm}{l}"] = rep(_f32(inputs[f"{nm}{l}"]))
        cat[f"loop_relT{l}"] = rep(_f32(np.asarray(inputs[f"loop_rel{l}"]).T))
        cat[f"b{l}"] = rep(_f32(np.asarray(inputs[f"b{l}"]).reshape(1, H)))
    return cat, spt


def cat_to_maps(cat, cfg: Cfg):
    """Slice concatenated arrays back into per-core input dicts (views)."""
    maps = []
    for c in range(cfg.n_cores):
        m = {}
        for k, v in cat.items():
            s0 = v.shape[0] // cfg.n_cores
            m[k] = v[c * s0 : (c + 1) * s0]
        maps.append(m)
    return maps


def build_nc(cfg: Cfg, spt: int, reps: int = 1):
    import concourse.bass as bass
    import concourse.tile as tile
    from concourse import bacc, mybir

    C = cfg.n_cores
    nt, nlp = cfg.nt, cfg.nlp
    RW = cfg.row_pad
    TPG = cfg.tiles_per_gather
    f32 = mybir.dt.float32
    bf16 = mybir.dt.bfloat16
    i32 = mybir.dt.int32
    Alu = mybir.AluOpType
    Act = mybir.ActivationFunctionType

    nc = bacc.Bacc(
        "TRN2", target_bir_lowering=False, debug=False, num_devices=C,
    )

    # ---- I/O declarations ----
    def din(name, shape, dt=f32):
        return nc.dram_tensor(name, list(shape), dt, kind="ExternalInput").ap()

    x_shard = din("x_shard", [nlp, H], bf16)
    gidx_d = din("gidx", [P, 2 * nt * spt], i32)
    dstrel_d = din("dstrel", [P, 2 * nt * spt], bf16)
    styp_d = din("styp", [P, 2 * nt * spt], bf16)
    dvalsrc_d = din("dvalsrc", [P, 2 * nt * spt], bf16)
    dval3_d = din("dval3", [P, 2 * nt * spt], bf16)
    dinv_own_d = din("dinv_own", [2, P, nt])
    batchrel_d = din("batchrel", [P, nt], bf16)
    invcnt_d = din("invcnt", [P, 2])
    onehotRT_d = din("onehotRT", [P, 512])
    rgT = din("rgT", [H, cfg.n_relg])
    tableT = din("tableT", [H, cfg.n_rel])
    lin1_d = din("lin1", [H, 2])
    lin2_d = din("lin2", [H, 2])
    lin_b_d = din("lin_b", [1, 2])
    Wd = {}
    for l in (1, 2, 3):
        for nm in ("w_in", "w_out", "w_loop", "w_rel"):
            Wd[f"{nm}{l}"] = din(f"{nm}{l}", [H, H])
        Wd[f"loop_relT{l}"] = din(f"loop_relT{l}", [H, 1])
        Wd[f"b{l}"] = din(f"b{l}", [1, H])

    out_d = nc.dram_tensor("out", [2 * P, 2], f32, kind="ExternalOutput").ap()

    xt_own = nc.dram_tensor("xt_own", [2 * nlp, RW], bf16).ap()
    xt_shared = nc.dram_tensor(
        "xt_shared", [C * 2 * nlp, RW], bf16, addr_space="Shared"
    ).ap()
    pool_own = nc.dram_tensor("pool_own", [P, 256], f32).ap()
    pool_shared = nc.dram_tensor("pool_shared", [P, 256], f32, addr_space="Shared").ap()

    groups = [list(range(C))]
    n_types = cfg.n_types  # 200
    tchunks = [(0, P), (P, n_types - P)] if n_types > P else [(0, n_types)]

    from concourse.masks import make_identity

    with tile.TileContext(nc) as tc:
        import contextlib

        ctx = contextlib.ExitStack()
        with ctx:
            cpool = ctx.enter_context(tc.tile_pool(name="consts", bufs=1))
            sbig = ctx.enter_context(tc.tile_pool(name="sbig", bufs=1))
            gpool = ctx.enter_context(tc.tile_pool(name="gath", bufs=3))
            mpool = ctx.enter_context(tc.tile_pool(name="mask", bufs=3))
            wpool = ctx.enter_context(tc.tile_pool(name="work", bufs=2))
            wconst = ctx.enter_context(tc.tile_pool(name="wconst", bufs=1))
            mtp = ctx.enter_context(tc.tile_pool(name="mts", bufs=2))
            pss = ctx.enter_context(tc.tile_pool(name="ps_s", bufs=2, space="PSUM"))
            psw = ctx.enter_context(tc.tile_pool(name="ps_w", bufs=1, space="PSUM"))
            pst = ctx.enter_context(tc.tile_pool(name="ps_t", bufs=2, space="PSUM"))
            dpool = ctx.enter_context(tc.tile_pool(name="dscr", bufs=1, space="DRAM"))
            m_scr = dpool.tile([512, nlp], bf16, tag="mscr")

            # ---- constants ----
            id_bf = cpool.tile([P, P], bf16)
            make_identity(nc, id_bf[:])
            iota128 = cpool.tile([P, P], bf16)
            nc.gpsimd.iota(iota128[:], pattern=[[1, P]], base=0,
                           channel_multiplier=0, allow_small_or_imprecise_dtypes=True)
            rowstg = [cpool.tile([P, 4, RW], bf16, tag="rowA", name="rowA"),
                      cpool.tile([P, 4, RW], bf16, tag="rowB", name="rowB")]
            nc.vector.memset(rowstg[0][:], 0.0)
            nc.vector.memset(rowstg[1][:], 0.0)
            iota256 = cpool.tile([P, 256], bf16)
            nc.gpsimd.iota(iota256[:], pattern=[[1, 256]], base=0,
                           channel_multiplier=0, allow_small_or_imprecise_dtypes=True)
            ones512 = cpool.tile([P, 512], f32)
            nc.vector.memset(ones512[:], 1.0)

            # SBUF-resident metadata
            gidx_sb = cpool.tile([P, 2 * nt * spt], i32)
            nc.sync.dma_start(gidx_sb[:], gidx_d[:])
            dstrel_sb = cpool.tile([P, 2 * nt * spt], bf16)
            nc.sync.dma_start(dstrel_sb[:], dstrel_d[:])
            styp_sb = cpool.tile([P, 2 * nt * spt], bf16)
            nc.sync.dma_start(styp_sb[:], styp_d[:])
            dvalsrc_sb = cpool.tile([P, 2 * nt * spt], bf16)
            nc.sync.dma_start(dvalsrc_sb[:], dvalsrc_d[:])
            dval3_sb = cpool.tile([P, 2 * nt * spt], bf16)
            nc.sync.dma_start(dval3_sb[:], dval3_d[:])
            dinv_own_sb = cpool.tile([P, 2 * nt], f32)
            nc.sync.dma_start(dinv_own_sb[:, :nt], dinv_own_d[0])
            nc.sync.dma_start(dinv_own_sb[:, nt:], dinv_own_d[1])
            batchrel_sb = cpool.tile([P, nt], bf16)
            nc.sync.dma_start(batchrel_sb[:], batchrel_d[:])

            # weights etc to SBUF
            Ws = {}
            for l in (1, 2, 3):
                for nm in ("w_in", "w_out", "w_loop", "w_rel"):
                    t = cpool.tile([H, H], f32, tag=f"{nm}{l}")
                    nc.sync.dma_start(t[:], Wd[f"{nm}{l}"][:])
                    Ws[f"{nm}{l}"] = t
                t = cpool.tile([H, 1], f32, tag=f"lrT{l}")
                nc.sync.dma_start(t[:], Wd[f"loop_relT{l}"][:])
                Ws[f"loop_relT{l}"] = t
                t = cpool.tile([P, H], f32, tag=f"b{l}")
                nc.sync.dma_start(t[:1, :], Wd[f"b{l}"][:])
                Ws[f"b{l}"] = t

            # rel_allT (f32, [H, n_types+1]) for layer 1
            relT = [None, None]  # double buffer across layers
            relT[0] = cpool.tile([H, n_types + 1], f32, tag="relA", name="relA")
            relT[1] = cpool.tile([H, n_types + 1], f32, tag="relB", name="relB")
            rgT_sb = cpool.tile([H, cfg.n_relg], f32)
            nc.sync.dma_start(rgT_sb[:], rgT[:])
            nc.vector.tensor_copy(relT[0][:, : cfg.n_relg], rgT_sb[:])
            nc.vector.tensor_scalar_mul(
                relT[0][:, cfg.n_relg : n_types], rgT_sb[:], -1.0
            )
            nc.vector.tensor_copy(relT[0][:, n_types : n_types + 1], Ws["loop_relT1"][:])

            # x_locT buffers (bf16 [H, nlp]) double buffered across layers
            xlt = [sbig.tile([H, nlp], bf16, tag="xltA", name="xltA"),
                   sbig.tile([H, nlp], bf16, tag="xltB", name="xltB")]
            at_in = sbig.tile([H, nt * P], bf16, tag="at_in")
            at_out = sbig.tile([H, nt * P], bf16, tag="at_out")

            import contextlib as _cl
            _loop = tc.For_i(0, reps, 1) if reps > 1 else _cl.nullcontext()
            with _loop:
                # ---------- prep stage: x~ from input x ----------
                def write_rows(src_tile_getter, layer_idx, nt_i, scaled):
                    """Scale node-major tile by per-dir dinv into a rotating
                    staging slot; one contiguous DMA per (tile, dir). Cols
                    128: stay zero from the one-time memset."""
                    sl = nt_i % 4
                    for d in range(2):
                        nc.vector.tensor_scalar(
                            rowstg[d][:, sl, :H], src_tile_getter(),
                            dinv_own_sb[:, d * nt + nt_i : d * nt + nt_i + 1], None,
                            op0=Alu.mult,
                        )
                        nc.sync.dma_start(
                            xt_own[d * nlp + nt_i * P : d * nlp + (nt_i + 1) * P, :],
                            rowstg[d][:, sl, :],
                        )

                def make_smask(d, g0, gn):
                    """Scaled one-hot dst mask: mask[slot, s, dst] =
                    (dstrel==dst) * dinv_dst/3 for the slots of tiles
                    [g0, g0+gn) of direction d."""
                    base = d * nt * spt + g0 * spt
                    mask = mpool.tile([P, TPG * spt, P], bf16, tag="mk")
                    nc.vector.tensor_tensor(
                        out=mask[:, : gn * spt, :],
                        in0=dstrel_sb[:, base : base + gn * spt]
                        .rearrange("p (t o) -> p t o", o=1)
                        .to_broadcast([P, gn * spt, P]),
                        in1=iota128[:]
                        .rearrange("p (o n) -> p o n", o=1)
                        .to_broadcast([P, gn * spt, P]),
                        op=Alu.is_equal,
                    )
                    nc.vector.tensor_tensor(
                        out=mask[:, : gn * spt, :],
                        in0=mask[:, : gn * spt, :],
                        in1=dval3_sb[:, base : base + gn * spt]
                        .rearrange("p (t o) -> p t o", o=1)
                        .to_broadcast([P, gn * spt, P]),
                        op=Alu.mult,
                    )
                    return mask

                # ---------- M' build (layer-invariant): M'[type, dst] =
                # sum_e dinv_src * dinv_dst/3 over edges of that type into
                # dst; staged via at_in/at_out, one DMA per (dir, chunk). ----
                for d in range(2):
                    stage = (at_in, at_out)
                    for g0 in range(0, nt, TPG):
                        gn = min(TPG, nt - g0)
                        base = d * nt * spt + g0 * spt
                        mask = make_smask(d, g0, gn)
                        wm = mpool.tile([P, TPG * spt, 2 * P], bf16, tag="wm")
                        nc.vector.tensor_tensor(
                            out=wm[:, : gn * spt, :],
                            in0=styp_sb[:, base : base + gn * spt]
                            .rearrange("p (t o) -> p t o", o=1)
                            .to_broadcast([P, gn * spt, 2 * P]),
                            in1=iota256[:]
                            .rearrange("p (o n) -> p o n", o=1)
                            .to_broadcast([P, gn * spt, 2 * P]),
                            op=Alu.is_equal,
                        )
                        nc.vector.tensor_tensor(
                            out=wm[:, : gn * spt, :],
                            in0=wm[:, : gn * spt, :],
                            in1=dvalsrc_sb[:, base : base + gn * spt]
                            .rearrange("p (t o) -> p t o", o=1)
                            .to_broadcast([P, gn * spt, 2 * P]),
                            op=Alu.mult,
                        )
                        for j in range(gn):
                            i = g0 + j
                            for ci in range(2):
                                psM = pss.tile([P, P], f32, tag="ps_s")
                                for s in range(spt):
                                    nc.tensor.matmul(
                                        out=psM[:],
                                        lhsT=wm[:, j * spt + s, ci * P : (ci + 1) * P],
                                        rhs=mask[:, j * spt + s, :],
                                        start=(s == 0),
                                        stop=(s == spt - 1),
                                    )
                                nc.scalar.copy(
                                    stage[ci][:, i * P : (i + 1) * P], psM[:]
                                )
                    nc.sync.dma_start(
                        m_scr[d * 256 : d * 256 + P, :], stage[0][:, : nt * P]
                    )
                    nc.sync.dma_start(
                        m_scr[d * 256 + P : d * 256 + 2 * P, :], stage[1][:, : nt * P]
                    )

                for i in range(nt):
                    xt_tile = wpool.tile([P, H], bf16, tag="xin")
                    nc.sync.dma_start(xt_tile[:], x_shard[i * P : (i + 1) * P, :])
                    # (a) x_locT
                    ps = pst.tile([P, P], bf16, tag="pst", name="pst")
                    nc.tensor.transpose(ps[:], xt_tile[:], id_bf[:])
                    nc.scalar.copy(xlt[0][:, i * P : (i + 1) * P], ps[:])
                    # (b) x~ rows
                    write_rows(lambda: xt_tile[:], 0, i, True)

                nc.gpsimd.collective_compute(
                    "AllGather", Alu.bypass, replica_groups=groups,
                    ins=[xt_own[:]], outs=[xt_shared[:]],
                )

                # ---------- layers ----------
                n_super = (nt + 3) // 4

                for l in (1, 2, 3):
                    cur, nxt = xlt[(l - 1) % 2], xlt[l % 2]
                    rel_cur = relT[(l - 1) % 2]
                    w_in, w_out = Ws[f"w_in{l}"], Ws[f"w_out{l}"]
                    w_loop, w_rel = Ws[f"w_loop{l}"], Ws[f"w_rel{l}"]

                    # --- per-layer small prep ---
                    wl3 = wconst.tile([H, H], f32, tag="wl3")
                    nc.vector.tensor_scalar_mul(wl3[:], w_loop[:], 1.0 / 3.0)
                    wl3_bf = wconst.tile([H, H], bf16, tag="wl3b")
                    nc.vector.tensor_copy(wl3_bf[:], wl3[:])
                    w_in_bf = wconst.tile([H, H], bf16, tag="winb")
                    nc.vector.tensor_copy(w_in_bf[:], w_in[:])
                    w_out_bf = wconst.tile([H, H], bf16, tag="woutb")
                    nc.vector.tensor_copy(w_out_bf[:], w_out[:])

                    # relw chunks (negated, bf16): dir-major chunk layout matches m_t
                    relwN = []
                    for d, w in ((0, w_in), (1, w_out)):
                        for (t0, tw) in tchunks:
                            psr = pst.tile([P, H], f32, tag="pst", name="pst")
                            nc.tensor.matmul(
                                out=psr[:tw, :], lhsT=rel_cur[:, t0 : t0 + tw],
                                rhs=w[:], start=True, stop=True,
                            )
                            rn = wconst.tile([P, H], bf16, tag=f"relw{d}{t0}")
                            nc.vector.memset(rn[:], 0.0)
                            nc.vector.tensor_scalar(
                                rn[:tw, :], psr[:tw, :], -1.0, None, op0=Alu.mult
                            )
                            relwN.append(rn)

                    # crow = b - (loop_rel @ w_loop)/3   [1, H] f32
                    psc = pst.tile([P, H], f32, tag="pst", name="pst")
                    nc.tensor.matmul(
                        out=psc[:1, :], lhsT=rel_cur[:, n_types : n_types + 1], rhs=wl3[:],
                        start=True, stop=True,
                    )
                    crow = wconst.tile([P, H], f32, tag="crow")
                    nc.vector.tensor_tensor(
                        out=crow[:1, :], in0=Ws[f"b{l}"][:1, :], in1=psc[:1, :],
                        op=Alu.subtract,
                    )

                    # rel evolution for next layer
                    if l < 3:
                        rel_nxt = relT[l % 2]
                        pse = pst.tile([P, n_types + 1], f32, tag="pst", name="pst")
                        nc.tensor.matmul(
                            out=pse[:, : n_types + 1], lhsT=w_rel[:],
                            rhs=rel_cur[:], start=True, stop=True,
                        )
                        nc.vector.tensor_copy(rel_nxt[:, :n_types], pse[:, :n_types])
                        nc.vector.tensor_copy(
                            rel_nxt[:, n_types : n_types + 1], Ws[f"loop_relT{l+1}"][:]
                        )

                    # --- S stage: per direction, per dst tile ---
                    for d in range(2):
                        at_buf = at_in if d == 0 else at_out
                        for g0 in range(0, nt, TPG):
                            gn = min(TPG, nt - g0)
                            gt = gpool.tile([P, TPG * spt, RW], bf16, tag="gt")
                            base = d * nt * spt + g0 * spt
                            for s in range(gn * spt):
                                nc.gpsimd.indirect_dma_start(
                                    out=gt[:, s, :],
                                    out_offset=None,
                                    in_=xt_shared[:],
                                    in_offset=bass.IndirectOffsetOnAxis(
                                        ap=gidx_sb[:, base + s : base + s + 1], axis=0
                                    ),
                                )
                            mask = make_smask(d, g0, gn)
                            for j in range(gn):
                                i = g0 + j
                                ps = pss.tile([P, P], f32, tag="ps_s")
                                for s in range(spt):
                                    nc.tensor.matmul(
                                        out=ps[:],
                                        lhsT=gt[:, j * spt + s, :H],
                                        rhs=mask[:, j * spt + s, :],
                                        start=(s == 0),
                                        stop=(s == spt - 1),
                                    )
                                nc.scalar.copy(
                                    at_buf[:, i * P : (i + 1) * P], ps[:]
                                )

                    # --- W stage (feature-major supertiles); the dinv_dst/3
                    # scaling is already folded into at_in/at_out/m_scr, so
                    # everything accumulates in a single PSUM tile. ---
                    for st in range(n_super):
                        c0 = st * 4 * P
                        W = min(4 * P, nt * P - c0)
                        ps = psw.tile([P, 4 * P], f32, tag="g1a")
                        nc.tensor.matmul(out=ps[:, :W], lhsT=w_in_bf[:],
                                         rhs=at_in[:, c0 : c0 + W], start=True, stop=False)
                        for half_i in range(2):
                            for ci, (t0, tw) in enumerate(tchunks):
                                mt = mtp.tile([P, 4 * P], bf16, tag="mt")
                                nc.sync.dma_start(
                                    mt[:, :W],
                                    m_scr[half_i * 256 + ci * P : half_i * 256 + (ci + 1) * P,
                                          c0 : c0 + W],
                                )
                                nc.tensor.matmul(
                                    out=ps[:, :W], lhsT=relwN[2 * half_i + ci][:],
                                    rhs=mt[:, :W], start=False, stop=False,
                                )
                            if half_i == 0:
                                nc.tensor.matmul(
                                    out=ps[:, :W], lhsT=w_out_bf[:],
                                    rhs=at_out[:, c0 : c0 + W], start=False, stop=False,
                                )
                        nc.tensor.matmul(out=ps[:, :W], lhsT=wl3_bf[:],
                                         rhs=cur[:, c0 : c0 + W], start=False, stop=False)
                        nc.tensor.matmul(out=ps[:, :W], lhsT=crow[:1, :],
                                         rhs=ones512[:1, :W], start=False, stop=True)
                        # tanh (+relu for l<3) -> nxt
                        th = wpool.tile([P, 4 * P], f32, tag="th")
                        nc.scalar.activation(th[:, :W], ps[:, :W], Act.Tanh)
                        if l < 3:
                            nc.vector.tensor_scalar_max(
                                nxt[:, c0 : c0 + W], th[:, :W], 0.0
                            )
                        else:
                            nc.vector.tensor_copy(nxt[:, c0 : c0 + W], th[:, :W])

                    # --- output rows / transposes ---
                    for i in range(nt):
                        pstr = pst.tile([P, P], bf16, tag="pst", name="pst")
                        nc.tensor.transpose(
                            pstr[:], nxt[:, i * P : (i + 1) * P], id_bf[:]
                        )
                        if l < 3:
                            write_rows(lambda: pstr[:], l, i, True)
                        else:
                            # keep node-major x3 in at_in buffer (free after W stage)
                            nc.vector.tensor_copy(
                                at_in[:, i * P : (i + 1) * P], pstr[:]
                            )

                    if l < 3:
                        nc.gpsimd.collective_compute(
                            "AllGather", Alu.bypass, replica_groups=groups,
                            ins=[xt_own[:]], outs=[xt_shared[:]],
                        )

                # ---------- pooling ----------
                psp = psw.tile([P, 256], f32, tag="pool")
                for i in range(nt):
                    oh = mpool.tile([P, 256], bf16, tag="ohb")
                    nc.vector.tensor_tensor(
                        out=oh[:],
                        in0=batchrel_sb[:, i : i + 1].to_broadcast([P, 256]),
                        in1=iota256[:],
                        op=Alu.is_equal,
                    )
                    nc.tensor.matmul(
                        out=psp[:], lhsT=at_in[:, i * P : (i + 1) * P], rhs=oh[:],
                        start=(i == 0), stop=(i == nt - 1),
                    )
                pooledT = wconst.tile([P, 256], f32, tag="pldT")
                nc.vector.tensor_copy(pooledT[:], psp[:])
                nc.sync.dma_start(pool_own[:], pooledT[:])
                nc.gpsimd.collective_compute(
                    "AllReduce", Alu.add, replica_groups=groups,
                    ins=[pool_own[:]], outs=[pool_shared[:]],
                )
                pooled_all = wconst.tile([P, 256], f32, tag="plda")
                nc.sync.dma_start(pooled_all[:], pool_shared[:])

                # ---------- head ----------
                lin1_sb = wconst.tile([H, 2], f32, tag="l1")
                nc.sync.dma_start(lin1_sb[:], lin1_d[:])
                lin2_sb = wconst.tile([H, 2], f32, tag="l2")
                nc.sync.dma_start(lin2_sb[:], lin2_d[:])
                linb_sb = wconst.tile([P, 2], f32, tag="lb")
                nc.sync.dma_start(linb_sb[:1, :], lin_b_d[:])
                invcnt_sb = wconst.tile([P, 2], f32, tag="ic")
                nc.sync.dma_start(invcnt_sb[:], invcnt_d[:])
                ones_col = wconst.tile([P, P], f32, tag="oc")
                nc.vector.memset(ones_col[:], 1.0)

                # tl2 = tableT.T @ lin2 -> [n_rel, 2], stored as 2 chunks side by side
                tableT_sb = wconst.tile([H, cfg.n_rel], f32, tag="tT")
                nc.sync.dma_start(tableT_sb[:], tableT[:])
                onehotRT_sb = wconst.tile([P, 512], f32, tag="ohr")
                nc.sync.dma_start(onehotRT_sb[:], onehotRT_d[:])
                rchunks = [(0, P), (P, cfg.n_rel - P)] if cfg.n_rel > P else [(0, cfg.n_rel)]
                tl2 = wconst.tile([P, 2 * 2], f32, tag="tl2")
                nc.vector.memset(tl2[:], 0.0)
                for ci, (t0, tw) in enumerate(rchunks):
                    pst2 = pst.tile([P, 2], f32, tag="pst", name="pst")
                    nc.tensor.matmul(
                        out=pst2[:tw, :], lhsT=tableT_sb[:, t0 : t0 + tw],
                        rhs=lin2_sb[:], start=True, stop=True,
                    )
                    nc.vector.tensor_copy(tl2[:tw, 2 * ci : 2 * ci + 2], pst2[:tw, :])

                for gc in range(2):
                    psA = pst.tile([P, 2], f32, tag="pst", name="pst")
                    nc.tensor.matmul(
                        out=psA[:], lhsT=pooled_all[:, gc * P : (gc + 1) * P],
                        rhs=lin1_sb[:], start=True, stop=True,
                    )
                    tA = wconst.tile([P, 2], f32, tag="tA")
                    nc.vector.tensor_scalar(
                        tA[:], psA[:], invcnt_sb[:, gc : gc + 1], None, op0=Alu.mult
                    )
                    psB = pst.tile([P, 2], f32, tag="pst", name="pst")
                    for ci, (t0, tw) in enumerate(rchunks):
                        nc.tensor.matmul(
                            out=psB[:],
                            lhsT=onehotRT_sb[:, ci * 256 + gc * P : ci * 256 + (gc + 1) * P],
                            rhs=tl2[:, 2 * ci : 2 * ci + 2],
                            start=(ci == 0), stop=False,
                        )
                    # lin_b via rank-1: out[g, c] += 1 * lin_b[c]
                    nc.tensor.matmul(
                        out=psB[:], lhsT=ones_col[:1, :], rhs=linb_sb[:1, :],
                        start=False, stop=True,
                    )
                    og = wconst.tile([P, 2], f32, tag="og")
                    nc.vector.tensor_tensor(
                        out=og[:], in0=tA[:], in1=psB[:], op=Alu.add
                    )
                    nc.sync.dma_start(out_d[gc * P : (gc + 1) * P, :], og[:])


    nc.compile()
    return nc


_CACHE = {}


def _make_runner(nc, n_cores: int):
    """Return a callable(cat_dict) -> list of np output arrays (global,
    concatenated over cores) that executes `nc` via a jitted shard_map.
    The jit is built once here; repeated calls reuse the compiled executable."""
    import jax
    from jax.experimental.shard_map import shard_map
    from jax.sharding import Mesh, PartitionSpec
    from concourse import bass2jax, mybir

    bass2jax.install_neuronx_cc_hook()

    partition_name = nc.partition_id_tensor.name if nc.partition_id_tensor else None
    in_names, out_names, out_avals, zero_shapes = [], [], [], []
    for alloc in nc.m.functions[0].allocations:
        if not isinstance(alloc, mybir.MemoryLocationSet):
            continue
        name = alloc.memorylocations[0].name
        if alloc.kind == "ExternalInput":
            if name != partition_name:
                in_names.append(name)
        elif alloc.kind == "ExternalOutput":
            out_names.append(name)
            shape = tuple(alloc.tensor_shape)
            dtype = mybir.dt.np(alloc.dtype)
            out_avals.append(jax.core.ShapedArray(shape, dtype))
            zero_shapes.append((shape, dtype))

    n_params = len(in_names)
    n_outs = len(out_avals)
    all_names = list(in_names) + list(out_names)
    if partition_name is not None:
        all_names.append(partition_name)
    donate = tuple(range(n_params, n_params + n_outs))

    def _body(*args):
        operands = list(args)
        if partition_name is not None:
            operands.append(bass2jax.partition_id_tensor())
        outs = bass2jax._bass_exec_p.bind(
            *operands,
            out_avals=tuple(out_avals),
            in_names=tuple(all_names),
            out_names=tuple(out_names),
            lowering_input_output_aliases=(),
            sim_require_finite=True,
            sim_require_nnan=True,
            nc=nc,
        )
        return tuple(outs)

    C = n_cores
    devices = jax.devices()[:C]
    mesh = Mesh(np.asarray(devices), ("core",))
    in_specs = (PartitionSpec("core"),) * (n_params + n_outs)
    out_specs = (PartitionSpec("core"),) * n_outs
    sharded = jax.jit(
        shard_map(_body, mesh=mesh, in_specs=in_specs, out_specs=out_specs,
                  check_rep=False),
        donate_argnums=donate,
        keep_unused=True,
    )

    def run(cat: dict) -> np.ndarray:
        ins = [cat[name] for name in in_names]
        zeros = [np.zeros((C * s[0],) + tuple(s[1:]), dt) for s, dt in zero_shapes]
        out_arrs = sharded(*ins, *zeros)
        i = out_names.index("out")
        per_core_rows = out_avals[i].shape[0]
        return np.asarray(out_arrs[i])[:per_core_rows]

    return run


def _get_runner(cfg: Cfg, spt: int):
    key = (cfg.n_nodes, cfg.n_edges, spt)
    if key not in _CACHE:
        _CACHE[key] = _make_runner(build_nc(cfg, spt), cfg.n_cores)
    return _CACHE[key]


def _run(inputs, cfg: Cfg, trace: bool = False):
    cat, spt = host_prepare_cat(inputs, cfg)
    run = _get_runner(cfg, spt)
    out = run(cat)[: cfg.n_graphs].astype(np.float32)
    return out, None


def kernel(**inputs) -> np.ndarray:
    cfg = Cfg()
    out, _ = _run(inputs, cfg)
    return out

